# revision 1
# baseline (speedup 1.0000x reference)
"""Trainium2 Bass kernel for a DiT-style transformer block (AdaLN + attention + SwiGLU MLP).

Sharding: sequence-parallel over 8 cores. Core c owns batch b=c//4, tokens
[512*(c%4), 512*(c%4)+512). K/V (RoPE'd, with per-head RMS scales) are
AllGather'd within each 4-core batch group; AdaLN mods are computed with the
contraction dim row-sharded across the group and AllReduce'd. Everything on
device is kept in transposed [feature, token] layout so no on-device
transposes are needed; matmuls run in float32r (full PE rate).
"""
import sys
sys.path.insert(0, '/opt/trn_rl_repo')

import numpy as np
import concourse.bass as bass
import concourse.tile as tile
from concourse import bacc, mybir

FP32 = mybir.dt.float32
FP32R = mybir.dt.float32r
AF = mybir.ActivationFunctionType

N_CORES = 8
B, T, D, H, DH = 2, 2048, 1024, 16, 64
HM = 2816
TOK = 512            # tokens per core
KT = D // 128        # 8 contraction tiles
HMT = HM // 128      # 22
EPS = 1e-6
KV_ROW = 1040 * 512  # flat size of each half of kv_in
KV_GROUPS = [[0, 1, 2, 3], [4, 5, 6, 7]]


def _ap(t, offset, dims):
    return bass.AP(tensor=t, offset=offset, ap=[list(d) for d in dims])


def build_program(reps=1, nocoll_tail=False):
    nc = bacc.Bacc("TRN2", target_bir_lowering=False, debug=False,
                   num_devices=N_CORES)

    xT = nc.declare_dram_parameter("xT", [D, TOK], FP32, isOutput=False)
    csh = nc.declare_dram_parameter("csh", [D, 1], FP32, isOutput=False)
    aw = nc.declare_dram_parameter("aw", [D, 6 * D], FP32, isOutput=False)
    ab = nc.declare_dram_parameter("ab", [6 * D], FP32, isOutput=False)
    qkvw = nc.declare_dram_parameter("qkvw", [D, 3 * D], FP32, isOutput=False)
    projw = nc.declare_dram_parameter("projw", [D, D], FP32, isOutput=False)
    projb = nc.declare_dram_parameter("projb", [D], FP32, isOutput=False)
    w1 = nc.declare_dram_parameter("w1", [D, HM], FP32, isOutput=False)
    w3 = nc.declare_dram_parameter("w3", [D, HM], FP32, isOutput=False)
    w2 = nc.declare_dram_parameter("w2", [HM, D], FP32, isOutput=False)
    cosq = nc.declare_dram_parameter("cosq", [128, TOK], FP32, isOutput=False)
    sinq = nc.declare_dram_parameter("sinq", [128, TOK], FP32, isOutput=False)
    cosk = nc.declare_dram_parameter("cosk", [128, TOK], FP32, isOutput=False)
    sink = nc.declare_dram_parameter("sink", [128, TOK], FP32, isOutput=False)
    perm = nc.declare_dram_parameter("perm", [128, 128], FP32, isOutput=False)
    consts = nc.declare_dram_parameter("consts", [128, 4], FP32, isOutput=False)
    outT = nc.declare_dram_parameter("outT", [D, TOK], FP32, isOutput=True)

    io = locals()
    with tile.TileContext(nc) as tc:
        for _rep in range(reps):
            _body(nc, tc, io, skip_collectives=(nocoll_tail and _rep > 0))
    nc.compile()
    return nc


def _body(nc, tc, io, skip_collectives=False):
    xT, csh, aw, ab = io["xT"], io["csh"], io["aw"], io["ab"]
    qkvw, projw, projb = io["qkvw"], io["projw"], io["projb"]
    w1, w3, w2 = io["w1"], io["w3"], io["w2"]
    cosq, sinq, cosk, sink = io["cosq"], io["sinq"], io["cosk"], io["sink"]
    perm, consts, outT = io["perm"], io["consts"], io["outT"]

    from contextlib import ExitStack
    ctx = ExitStack()
    # pools alive for the whole kernel
    pp = ctx.enter_context(tc.tile_pool(name="persist", bufs=1))
    dr = ctx.enter_context(tc.tile_pool(name="dram", bufs=1, space="DRAM"))

    # ---------- global constants ----------
    consts_sb = pp.tile([128, 4], FP32R, name="consts_sb")
    nc.sync.dma_start(out=consts_sb, in_=consts[:, :].bitcast(FP32R))
    xT_sb = pp.tile([128, KT, TOK], FP32, name="xT_sb")
    nc.sync.dma_start(out=xT_sb, in_=xT.rearrange("(kt p) t -> p kt t", p=128))
    projb_sb = pp.tile([128, 8], FP32, name="projb_sb")
    nc.sync.dma_start(out=projb_sb, in_=projb.rearrange("(oc p) -> p oc", p=128))
    eps1 = pp.tile([128, 1], FP32, name="eps1")
    nc.vector.memset(eps1, EPS)
    epsd = pp.tile([128, 1], FP32, name="epsd")
    nc.vector.memset(epsd, DH * EPS)
    mods_sb = pp.tile([128, 6, 8], FP32, name="mods_sb")
    s1p_msa = pp.tile([128, 8], FP32, name="s1p_msa")
    s1p_mlp = pp.tile([128, 8], FP32, name="s1p_mlp")
    bgp = pp.tile([128, 8], FP32, name="bgp")
    x1T = pp.tile([128, KT, TOK], FP32, name="x1T")
    perm_sb = pp.tile([128, 128], FP32R, name="perm_sb")
    nc.sync.dma_start(out=perm_sb, in_=perm[:, :].bitcast(FP32R))

    # ---------- DRAM scratch ----------
    mods_in = dr.tile([6 * D], FP32, name="mods_in")
    kv_in = dr.tile([2, KV_ROW], FP32, name="kv_in")
    kv_out = dr.tile([4, 2, KV_ROW], FP32, name="kv_out")

    def rms_rb(pool_ps, pool_t, pool_s, src_tile3, scale, bias, tag):
        """token-wise 1/sqrt(mean(sq)+eps) over the partition (feature) dim,
        broadcast to all 128 partitions"""
        ps_ssq = pool_ps.tile([1, 512], FP32, tag=f"ps_ssq_{tag}", bufs=1,
                              name=f"ps_ssq_{tag}")
        for kt in range(KT):
            xsq = pool_t.tile([128, TOK], FP32R, tag=f"sq_{tag}", bufs=2,
                              name=f"sq_{tag}")
            nc.scalar.activation(out=xsq, in_=src_tile3[:, kt, :], func=AF.Square)
            nc.tensor.matmul(ps_ssq, consts_sb[:, 0:1], xsq,
                             start=(kt == 0), stop=(kt == KT - 1))
        rt = pool_s.tile([1, 512], FP32, tag=f"rt_{tag}", name=f"rt_{tag}")
        nc.scalar.activation(out=rt, in_=ps_ssq, func=AF.Sqrt,
                             scale=scale, bias=bias)
        rv = pool_s.tile([1, 512], FP32, tag=f"rv_{tag}", name=f"rv_{tag}")
        nc.vector.reciprocal(rv, rt)
        rb = pool_t.tile([128, TOK], FP32, tag=f"rb_{tag}", bufs=1,
                         name=f"rb_{tag}")
        nc.gpsimd.partition_broadcast(rb, rv)
        return rb

    # =========================================================
    # Scope AB: qn lives from P3 into attention
    # =========================================================
    ab_pool = ctx.enter_context(tc.tile_pool(name="scope_ab", bufs=1))
    qn = ab_pool.tile([128, 8, TOK], FP32R, name="qn")

    with tc.tile_pool(name="sA", bufs=1) as pa, \
         tc.tile_pool(name="wA", bufs=2) as wp, \
         tc.tile_pool(name="tA", bufs=2) as tp, \
         tc.tile_pool(name="smA", bufs=1) as sp:

        # ---------- P0: AdaLN mods ----------
        with tc.tile_pool(name="psA1", bufs=1, space="PSUM") as ps1:
            csh_sb = pa.tile([128, 8, 1], FP32, name="csh_sb")
            nc.sync.dma_start(out=csh_sb,
                              in_=csh.rearrange("(kt p) o -> p kt o", p=128))
            silu_c = pa.tile([128, 8, 1], FP32R, name="silu_c")
            nc.scalar.activation(out=silu_c, in_=csh_sb, func=AF.Silu)
            for ncn in range(12):
                aw_t = wp.tile([128, 8, 512], FP32R, tag="aw_t", name="aw_t")
                nc.sync.dma_start(
                    out=aw_t,
                    in_=aw[:, 512 * ncn:512 * (ncn + 1)]
                    .rearrange("(kt p) n -> p kt n", p=128).bitcast(FP32R))
                ps_m = ps1.tile([1, 512], FP32, tag="ps_mods", bufs=2,
                                name="ps_m")
                for kt in range(8):
                    nc.tensor.matmul(ps_m, silu_c[:, kt, :], aw_t[:, kt, :],
                                     start=(kt == 0), stop=(kt == 7))
                stg = sp.tile([1, 512], FP32, tag="mods_stg", name="stg")
                nc.scalar.copy(stg, ps_m)
                nc.sync.dma_start(
                    out=_ap(mods_in.tensor, mods_in.offset + 512 * ncn,
                            [[512, 1], [1, 512]]),
                    in_=stg)
            nc.sync.dma_start(
                out=mods_sb,
                in_=mods_in.rearrange("(v kt p) -> p v kt", p=128, kt=8))
            ab_sb = pa.tile([128, 6, 8], FP32, name="ab_sb")
            nc.sync.dma_start(
                out=ab_sb, in_=ab.rearrange("(v kt p) -> p v kt", p=128, kt=8))
            nc.vector.tensor_add(mods_sb, mods_sb, ab_sb)
            nc.scalar.add(s1p_msa, mods_sb[:, 1, :], 1.0)
            nc.scalar.add(s1p_mlp, mods_sb[:, 4, :], 1.0)
            nc.vector.tensor_mul(bgp, projb_sb, mods_sb[:, 2, :])

            # ---------- P1: x_modT ----------
            rb1 = rms_rb(ps1, tp, sp, xT_sb, 1.0 / D, eps1[0:1, :], "n1")
            x_modT = pa.tile([128, KT, TOK], FP32R, name="x_modT")
            for kt in range(KT):
                xr = tp.tile([128, TOK], FP32, tag="xr1", name="xr")
                nc.vector.tensor_mul(xr, xT_sb[:, kt, :], rb1)
                nc.vector.tensor_scalar(
                    out=x_modT[:, kt, :], in0=xr,
                    scalar1=s1p_msa[:, kt:kt + 1],
                    scalar2=mods_sb[:, 0, kt:kt + 1],
                    op0=mybir.AluOpType.mult, op1=mybir.AluOpType.add)

        # ---------- P2/P3: q/k/v projections, rope, kv allgather ----------
        cosq_sb = pa.tile([128, TOK], FP32, name="cosq_sb")
        sinq_sb = pa.tile([128, TOK], FP32, name="sinq_sb")
        cosk_sb = pa.tile([128, TOK], FP32, name="cosk_sb")
        sink_sb = pa.tile([128, TOK], FP32, name="sink_sb")
        nc.sync.dma_start(out=cosq_sb, in_=cosq[:, :])
        nc.sync.dma_start(out=sinq_sb, in_=sinq[:, :])
        nc.sync.dma_start(out=cosk_sb, in_=cosk[:, :])
        nc.sync.dma_start(out=sink_sb, in_=sink[:, :])

        with tc.tile_pool(name="psA2", bufs=1, space="PSUM") as ps2:

            def proj_T(col0, oc):
                w_t = wp.tile([128, KT, 128], FP32R, tag="w_pT", name="w_t")
                nc.sync.dma_start(
                    out=w_t,
                    in_=qkvw[:, col0 + 128 * oc: col0 + 128 * (oc + 1)]
                    .rearrange("(kt p) n -> p kt n", p=128).bitcast(FP32R))
                ps_p = ps2.tile([128, 512], FP32, tag="ps_pT", bufs=2,
                                name="ps_p")
                for kt in range(KT):
                    nc.tensor.matmul(ps_p, w_t[:, kt, :], x_modT[:, kt, :],
                                     start=(kt == 0), stop=(kt == KT - 1))
                return ps_p

            def rope(ps_raw, cos_sb, sin_sb, tag):
                raw = tp.tile([128, TOK], FP32R, tag=f"raw_{tag}", bufs=2,
                              name="raw")
                nc.vector.tensor_copy(out=raw, in_=ps_raw)
                ps_sh = ps2.tile([128, 512], FP32, tag="ps_sh", bufs=2,
                                 name="ps_sh")
                nc.tensor.matmul(ps_sh, perm_sb, raw, start=True, stop=True)
                t1 = tp.tile([128, TOK], FP32, tag=f"t1_{tag}", bufs=2,
                             name="t1")
                nc.vector.tensor_mul(t1, raw, cos_sb)
                t2 = tp.tile([128, TOK], FP32, tag=f"t2_{tag}", bufs=2,
                             name="t2")
                nc.vector.tensor_mul(t2, ps_sh, sin_sb)
                return raw, t1, t2

            def head_rms(raw, scale, bias, tag):
                sq = tp.tile([128, TOK], FP32R, tag=f"hsq_{tag}", bufs=2,
                             name="sq")
                nc.scalar.activation(out=sq, in_=raw, func=AF.Square)
                ps_h = ps2.tile([2, 512], FP32, tag="ps_h", bufs=2,
                                name="ps_h")
                nc.tensor.matmul(ps_h, consts_sb[:, 1:3], sq,
                                 start=True, stop=True)
                rs = sp.tile([2, 512], FP32, tag=f"rs_{tag}", name="rs")
                nc.scalar.activation(out=rs, in_=ps_h, func=AF.Sqrt,
                                     scale=scale, bias=bias)
                rvv = sp.tile([2, 512], FP32, tag=f"rvv_{tag}", name="rvv")
                nc.vector.reciprocal(rvv, rs)
                return rvv

            for oc in range(8):
                ps_k = proj_T(D, oc)
                raw, t1, t2 = rope(ps_k, cosk_sb, sink_sb, "k")
                rkv = head_rms(raw, 1.0 / DH, eps1[0:2, :], "k")
                nc.sync.dma_start(
                    out=_ap(kv_in.tensor,
                            kv_in.offset + (1024 + 2 * oc) * 512,
                            [[512, 2], [1, 512]]),
                    in_=rkv)
                kn = tp.tile([128, TOK], FP32, tag="kn", bufs=2, name="kn")
                nc.vector.tensor_add(kn, t1, t2)
                nc.sync.dma_start(
                    out=_ap(kv_in.tensor, kv_in.offset + 128 * oc * 512,
                            [[512, 128], [1, 512]]),
                    in_=kn)

            for ncn in range(2):
                wv_t = wp.tile([128, KT, 512], FP32R, tag="wv_t", bufs=1, name="wv_t")
                nc.sync.dma_start(
                    out=wv_t,
                    in_=qkvw[:, 2 * D + 512 * ncn: 2 * D + 512 * (ncn + 1)]
                    .rearrange("(kt p) n -> p kt n", p=128).bitcast(FP32R))
                for mt in range(4):
                    vaug = tp.tile([128, 8, 65], FP32, tag="vaug", bufs=2,
                                   name="vaug")
                    nc.vector.memset(vaug[:, :, 64:65], 1.0)
                    ps_v = ps2.tile([128, 512], FP32, tag="ps_pT", bufs=2,
                                    name="ps_v")
                    for kt in range(KT):
                        nc.tensor.matmul(
                            ps_v, x_modT[:, kt, 128 * mt:128 * (mt + 1)],
                            wv_t[:, kt, :],
                            start=(kt == 0), stop=(kt == KT - 1))
                    nc.scalar.copy(vaug[:, :, 0:64],
                                   ps_v.rearrange("p (h d) -> p h d", d=64))
                    nc.sync.dma_start(
                        out=_ap(kv_in.tensor,
                                kv_in.offset + KV_ROW + 128 * mt * 1040
                                + 65 * 8 * ncn,
                                [[1040, 128], [1, 520]]),
                        in_=vaug.rearrange("p h d -> p (h d)"))

            if not skip_collectives:
                nc.gpsimd.collective_compute(
                    "AllGather", mybir.AluOpType.bypass,
                    replica_groups=KV_GROUPS,
                    ins=[kv_in.rearrange("a b -> (a b)")],
                    outs=[kv_out.rearrange("a b c -> (a b c)")])

            # ---------- P3: qT ----------
            for oc in range(8):
                ps_q = proj_T(0, oc)
                raw, t1, t2 = rope(ps_q, cosq_sb, sinq_sb, "q")
                rqv = head_rms(raw, 1.0, epsd[0:2, :], "q")
                rqd = dr.tile([2, 512], FP32, tag="rqd", bufs=2, name="rqd")
                nc.sync.dma_start(out=rqd, in_=rqv)
                rqb = tp.tile([128, TOK], FP32, tag="rqb", bufs=2, name="rqb")
                nc.sync.dma_start(
                    out=rqb[0:64, :],
                    in_=_ap(rqd.tensor, rqd.offset, [[0, 64], [1, 512]]))
                nc.sync.dma_start(
                    out=rqb[64:128, :],
                    in_=_ap(rqd.tensor, rqd.offset + 512, [[0, 64], [1, 512]]))
                t3 = tp.tile([128, TOK], FP32, tag="t3_q", bufs=2, name="t3")
                nc.vector.tensor_add(t3, t1, t2)
                nc.vector.tensor_mul(qn[:, oc, :], t3, rqb)


    # =========================================================
    # Scope B: attention + proj
    # =========================================================
    with tc.tile_pool(name="sB", bufs=1) as pb, \
         tc.tile_pool(name="wB", bufs=2) as wpb, \
         tc.tile_pool(name="tB", bufs=2) as tpb, \
         tc.tile_pool(name="smB", bufs=1) as spb, \
         tc.tile_pool(name="psB", bufs=1, space="PSUM") as psb:

        attn_oT = []
        for h in range(H):
            aot = pb.tile([64, TOK], FP32R, name=f"attn_oT{h}")
            attn_oT.append(aot)

        kv_t = kv_out.tensor
        kv_off = kv_out.offset
        for hp in range(8):
            kn_pair = wpb.tile([128, 4, 512], FP32R, tag="kn_pair", bufs=3,
                               name="kn_pair")
            nc.sync.dma_start(
                out=kn_pair,
                in_=_ap(kv_t, kv_off + 128 * hp * 512,
                        [[512, 128], [2 * KV_ROW, 4], [1, 512]])
                .bitcast(FP32R))
            rk_sb = spb.tile([128, 2, 4, 4], FP32, tag="rk_sb", bufs=2,
                             name="rk_sb")
            for hh2 in range(2):
                for s2 in range(4):
                    nc.sync.dma_start(
                        out=rk_sb[:, hh2, s2, :],
                        in_=_ap(kv_t,
                                kv_off + 2 * s2 * KV_ROW
                                + (1024 + 2 * hp + hh2) * 512,
                                [[1, 128], [128, 4]]))
            vaug_h = []
            for hh in range(2):
                h = 2 * hp + hh
                vh = wpb.tile([128, 4, 4, 65], FP32R, tag=f"vaug_h{hh}",
                              bufs=3, name=f"vaug_h{hh}")
                for s2 in range(4):
                    nc.sync.dma_start(
                        out=vh[:, s2, :, :],
                        in_=_ap(kv_t, kv_off + (2 * s2 + 1) * KV_ROW + 65 * h,
                                [[1040, 128], [128 * 1040, 4], [1, 65]])
                        .bitcast(FP32R))
                vaug_h.append(vh)
            ps_o = []
            for hh in range(2):
                pso = psb.tile([65, 512], FP32, tag=f"ps_o{hh}", bufs=1,
                               name=f"ps_o{hh}")
                ps_o.append(pso)
            for su in range(16):
                s, u = su // 4, su % 4
                for hh in range(2):
                    ps_s = psb.tile([128, 512], FP32, tag=f"ps_s{hh}", bufs=2,
                                    name=f"ps_s{hh}")
                    nc.tensor.matmul(
                        ps_s,
                        kn_pair[64 * hh:64 * (hh + 1), s,
                                128 * u:128 * (u + 1)],
                        qn[64 * hh:64 * (hh + 1), hp, :],
                        start=True, stop=True, tile_position=(64 * hh, 0))
                    exp_t = tpb.tile([128, 512], FP32R, tag=f"exp{hh}", bufs=4,
                                     name=f"exp{hh}")
                    nc.scalar.activation(out=exp_t, in_=ps_s, func=AF.Exp,
                                         scale=rk_sb[:, hh, s, u:u + 1])
                    nc.tensor.matmul(ps_o[hh], vaug_h[hh][:, s, u, :], exp_t,
                                     start=(su == 0), stop=(su == 15))
            for hh in range(2):
                h = 2 * hp + hh
                rd = spb.tile([1, 512], FP32, tag=f"rd{hh}", name=f"rd{hh}")
                nc.vector.reciprocal(rd, ps_o[hh][64:65, :])
                rdb = tpb.tile([64, 512], FP32, tag=f"rdb{hh}", bufs=2,
                               name=f"rdb{hh}")
                nc.gpsimd.partition_broadcast(rdb, rd)
                nc.vector.tensor_mul(attn_oT[h], ps_o[hh][0:64, :], rdb)

        # ---------- P5: proj + gated residual -> x1T ----------
        for oc in range(8):
            wproj_t = wpb.tile([64, 16, 128], FP32R, tag="wproj_t", bufs=3,
                               name="wproj_t")
            nc.sync.dma_start(
                out=wproj_t,
                in_=projw[:, 128 * oc:128 * (oc + 1)]
                .rearrange("(h p) m -> p h m", p=64).bitcast(FP32R))
            ps_p = psb.tile([128, 512], FP32, tag="ps_proj", bufs=2,
                            name="ps_p")
            for h in range(H):
                nc.tensor.matmul(ps_p, wproj_t[:, h, :], attn_oT[h],
                                 start=(h == 0), stop=(h == H - 1))
            t1 = tpb.tile([128, TOK], FP32, tag="t1_proj", bufs=2, name="t1")
            nc.vector.tensor_scalar(
                out=t1, in0=ps_p,
                scalar1=mods_sb[:, 2, oc:oc + 1], scalar2=bgp[:, oc:oc + 1],
                op0=mybir.AluOpType.mult, op1=mybir.AluOpType.add)
            nc.vector.tensor_add(x1T[:, oc, :], t1, xT_sb[:, oc, :])

    # =========================================================
    # Scope C: norm2 + MLP
    # =========================================================
    with tc.tile_pool(name="sC", bufs=1) as pc, \
         tc.tile_pool(name="wC", bufs=2) as wpc, \
         tc.tile_pool(name="tC", bufs=2) as tpc, \
         tc.tile_pool(name="smC", bufs=1) as spc, \
         tc.tile_pool(name="psC", bufs=1, space="PSUM") as psc:

        rb2 = rms_rb(psc, tpc, spc, x1T, 1.0 / D, eps1[0:1, :], "n2")
        x1_modT = pc.tile([128, KT, TOK], FP32R, name="x1_modT")
        for kt in range(KT):
            xr2 = tpc.tile([128, TOK], FP32, tag="xr2", name="xr2")
            nc.vector.tensor_mul(xr2, x1T[:, kt, :], rb2)
            nc.vector.tensor_scalar(
                out=x1_modT[:, kt, :], in0=xr2,
                scalar1=s1p_mlp[:, kt:kt + 1],
                scalar2=mods_sb[:, 3, kt:kt + 1],
                op0=mybir.AluOpType.mult, op1=mybir.AluOpType.add)

        mT = pc.tile([128, HMT, TOK], FP32R, name="mT")
        for hm in range(HMT):
            w1_t = wpc.tile([128, KT, 128], FP32R, tag="w1_t", bufs=3, name="w1_t")
            nc.sync.dma_start(
                out=w1_t,
                in_=w1[:, 128 * hm:128 * (hm + 1)]
                .rearrange("(kt p) n -> p kt n", p=128).bitcast(FP32R))
            w3_t = wpc.tile([128, KT, 128], FP32R, tag="w3_t", bufs=3, name="w3_t")
            nc.sync.dma_start(
                out=w3_t,
                in_=w3[:, 128 * hm:128 * (hm + 1)]
                .rearrange("(kt p) n -> p kt n", p=128).bitcast(FP32R))
            ps_u = psc.tile([128, 512], FP32, tag="ps_u", bufs=2, name="ps_u")
            ps_g = psc.tile([128, 512], FP32, tag="ps_g", bufs=2, name="ps_g")
            for kt in range(KT):
                nc.tensor.matmul(ps_u, w1_t[:, kt, :], x1_modT[:, kt, :],
                                 start=(kt == 0), stop=(kt == KT - 1))
            for kt in range(KT):
                nc.tensor.matmul(ps_g, w3_t[:, kt, :], x1_modT[:, kt, :],
                                 start=(kt == 0), stop=(kt == KT - 1))
            tsil = tpc.tile([128, TOK], FP32, tag="tsil", name="tsil")
            nc.scalar.activation(out=tsil, in_=ps_u, func=AF.Silu)
            nc.vector.tensor_mul(mT[:, hm, :], tsil, ps_g)

        for oc in range(8):
            w2_t = wpc.tile([128, HMT, 128], FP32R, tag="w2_t", bufs=3, name="w2_t")
            nc.sync.dma_start(
                out=w2_t,
                in_=w2[:, 128 * oc:128 * (oc + 1)]
                .rearrange("(hm p) n -> p hm n", p=128).bitcast(FP32R))
            ps_w2 = psc.tile([128, 512], FP32, tag="ps_w2", bufs=2,
                             name="ps_w2")
            for hm in range(HMT):
                nc.tensor.matmul(ps_w2, w2_t[:, hm, :], mT[:, hm, :],
                                 start=(hm == 0), stop=(hm == HMT - 1))
            t3 = tpc.tile([128, TOK], FP32, tag="t3_out", bufs=2, name="t3")
            nc.vector.tensor_scalar(
                out=t3, in0=ps_w2,
                scalar1=mods_sb[:, 5, oc:oc + 1], scalar2=None,
                op0=mybir.AluOpType.mult)
            outf = tpc.tile([128, TOK], FP32, tag="outf", bufs=2, name="outf")
            nc.vector.tensor_add(outf, t3, x1T[:, oc, :])
            nc.sync.dma_start(out=outT[128 * oc:128 * (oc + 1), :], in_=outf)

    ctx.close()


# ------------------------------------------------------------------
# host side
# ------------------------------------------------------------------

def _host_tables(pos, lnq_w, lnk_w):
    half = DH // 2
    freqs = (1.0 / (10000.0 ** (np.arange(half, dtype=np.float32) / half))
             ).astype(np.float32)
    ang = pos.astype(np.float32)[:, None] * freqs[None, :]      # [T, 32]
    cos2 = np.concatenate([np.cos(ang), np.cos(ang)], -1).astype(np.float32)
    sin2 = np.concatenate([np.sin(ang), np.sin(ang)], -1).astype(np.float32)
    shufsrc = np.concatenate([np.arange(32) + 32, np.arange(32)])
    cosF_q = cos2 * lnq_w[None, :]
    sinF_q = sin2 * lnq_w[shufsrc][None, :]
    cosF_k = cos2 * lnk_w[None, :]
    sinF_k = sin2 * lnk_w[shufsrc][None, :]

    P = np.zeros((128, 128), np.float32)
    for blk in (0, 64):
        for m in range(64):
            P[blk + shufsrc[m], blk + m] = -1.0 if m < 32 else 1.0

    consts = np.zeros((128, 4), np.float32)
    consts[:, 0] = 1.0
    consts[0:64, 1] = 1.0
    consts[64:128, 2] = 1.0
    return cosF_q, sinF_q, cosF_k, sinF_k, P, consts


def _prep_in_maps(inputs):
    x = np.asarray(inputs["x"], np.float32)
    c = np.asarray(inputs["c"], np.float32)
    pos = np.asarray(inputs["pos"])
    cosF_q, sinF_q, cosF_k, sinF_k, P, consts = _host_tables(
        pos, np.asarray(inputs["lnq_w"], np.float32),
        np.asarray(inputs["lnk_w"], np.float32))
    shared = {
        "ab": np.ascontiguousarray(inputs["adaln_b"], np.float32),
        "qkvw": np.ascontiguousarray(inputs["qkv_w"], np.float32),
        "projw": np.ascontiguousarray(inputs["proj_w"], np.float32),
        "projb": np.ascontiguousarray(inputs["proj_b"], np.float32),
        "w1": np.ascontiguousarray(inputs["w1_w"], np.float32),
        "w3": np.ascontiguousarray(inputs["w3_w"], np.float32),
        "w2": np.ascontiguousarray(inputs["w2_w"], np.float32),
        "perm": P, "consts": consts,
    }
    adaln_w = np.asarray(inputs["adaln_w"], np.float32)
    in_maps = []
    for core in range(N_CORES):
        b, ti = core // 4, core % 4
        q0 = TOK * ti
        tile2 = lambda a: np.ascontiguousarray(
            np.tile(a[q0:q0 + TOK].T, (2, 1)))  # [64,512] -> [128,512]
        m = dict(shared)
        m["xT"] = np.ascontiguousarray(x[b, q0:q0 + TOK, :].T)
        m["csh"] = np.ascontiguousarray(c[b]).reshape(D, 1)
        m["aw"] = adaln_w
        m["cosq"] = tile2(cosF_q)
        m["sinq"] = tile2(sinF_q)
        m["cosk"] = tile2(cosF_k)
        m["sink"] = tile2(sinF_k)
        in_maps.append(m)
    return in_maps


_RUNNER = {}


def _get_runner(reps=1, nocoll_tail=False):
    global _RUNNER
    key = (reps, nocoll_tail)
    if key in _RUNNER:
        return _RUNNER[key]
    import jax
    from jax.sharding import Mesh, PartitionSpec
    from jax.experimental.shard_map import shard_map
    from concourse import bass2jax, mybir as _mybir

    nc = build_program(reps, nocoll_tail)
    bass2jax.install_neuronx_cc_hook()

    partition_name = (nc.partition_id_tensor.name
                      if nc.partition_id_tensor else None)
    in_names, out_names, out_avals, zero_outs = [], [], [], []
    for alloc in nc.m.functions[0].allocations:
        if not isinstance(alloc, _mybir.MemoryLocationSet):
            continue
        name = alloc.memorylocations[0].name
        if alloc.kind == "ExternalInput":
            if name != partition_name:
                in_names.append(name)
        elif alloc.kind == "ExternalOutput":
            shape = tuple(alloc.tensor_shape)
            dtype = _mybir.dt.np(alloc.dtype)
            out_names.append(name)
            out_avals.append(jax.core.ShapedArray(shape, dtype))
            zero_outs.append(np.zeros(shape, dtype))
    n_params = len(in_names)
    n_outs = len(out_avals)
    all_names = in_names + out_names
    if partition_name is not None:
        all_names = all_names + [partition_name]
    donate = tuple(range(n_params, n_params + n_outs))

    def _bd(*args):
        operands = list(args)
        if partition_name is not None:
            operands.append(bass2jax.partition_id_tensor())
        outs = bass2jax._bass_exec_p.bind(
            *operands, out_avals=tuple(out_avals), in_names=tuple(all_names),
            out_names=tuple(out_names), lowering_input_output_aliases=(),
            sim_require_finite=True, sim_require_nnan=True, nc=nc)
        return tuple(outs)

    devices = jax.devices()[:N_CORES]
    mesh = Mesh(np.asarray(devices), ("core",))
    sharded = jax.jit(
        shard_map(_bd, mesh=mesh,
                  in_specs=(PartitionSpec("core"),) * (n_params + n_outs),
                  out_specs=(PartitionSpec("core"),) * n_outs,
                  check_rep=False),
        donate_argnums=donate, keep_unused=True)

    def run(in_maps):
        concat_in = [np.concatenate([np.asarray(m[nm]) for m in in_maps], 0)
                     for nm in in_names]
        concat_zeros = [np.zeros((N_CORES * z.shape[0], *z.shape[1:]), z.dtype)
                        for z in zero_outs]
        out_arrs = sharded(*concat_in, *concat_zeros)
        return [
            {nm: np.asarray(out_arrs[i]).reshape(N_CORES, *out_avals[i].shape)[cc]
             for i, nm in enumerate(out_names)}
            for cc in range(N_CORES)
        ]

    def bench(in_maps, iters_lo=4, iters_hi=24):
        import time as _time
        concat_in = [np.concatenate([np.asarray(m[nm]) for m in in_maps], 0)
                     for nm in in_names]
        dev_in = [jax.device_put(a) for a in concat_in]
        for a in dev_in:
            a.block_until_ready()

        def zero_set():
            zs = [jax.device_put(
                np.zeros((N_CORES * z.shape[0], *z.shape[1:]), z.dtype))
                for z in zero_outs]
            for z in zs:
                z.block_until_ready()
            return zs

        out = sharded(*dev_in, *zero_set())  # warm-up
        for o in out:
            o.block_until_ready()

        results = {}
        for iters in (iters_lo, iters_hi):
            staged = [zero_set() for _ in range(iters)]
            t0 = _time.time()
            out = None
            for i in range(iters):
                out = sharded(*dev_in, *staged[i])
            for o in out:
                o.block_until_ready()
            results[iters] = _time.time() - t0
        per_iter = (results[iters_hi] - results[iters_lo]) / (iters_hi - iters_lo)
        return per_iter, results

    run.bench = bench
    _RUNNER[key] = run
    return run


def kernel(**inputs) -> np.ndarray:
    run = _get_runner()
    in_maps = _prep_in_maps(inputs)
    results = run(in_maps)
    out = np.empty((B, T, D), np.float32)
    for core in range(N_CORES):
        b, ti = core // 4, core % 4
        out[b, TOK * ti:TOK * (ti + 1), :] = results[core]["outT"].T
    return out

